# revision 18
# baseline (speedup 1.0000x reference)
"""Trainium2 Bass kernel for nn_Attention_6201932775733 (sparse window attention).

v2 design (8 NeuronCores, SPMD, no collectives):
  - Data-parallel over (batch, 16-row stripe blocks): core i handles batch
    i//4, x-blocks {2*(i%4), 2*(i%4)+1}; positions pre-permuted window-major.
  - Work is pipelined over 8 "units" per core (stripe x 512-position block =
    2 windows x 8 heads), with projection(k), attention(k-1), out-proj(k-2)
    emitted skewed so all engines overlap.
  - Q/K channel layout is split rot/pass: chunks [C0 C1] = rotary channels
    (heads 0-3 / 4-7, 32 each), [C2 C3] = pass-through channels. Pass chunks
    evict PSUM->SBUF with a plain copy; only rot chunks pay the cos/sin
    multiply-add. RoPE rotation weights are packed (no zero rows).
  - sim per (window, head) = K=32 matmuls (rot + pass accumulate); heads in
    a triple target distinct 32-row PE groups so their matmuls overlap.
  - AV uses a ones-column per head so the softmax denominator is row 64 of
    the [65, 512] window-pair PSUM tile; denominators round-trip through two
    reshaping DMAs for a partition-parallel reciprocal, then a K=1 matmul
    broadcasts 1/d and one DVE multiply normalizes into the bf16 u slab.
  - Everything on the PE path is bf16 (host pre-converts inputs/weights);
    PSUM accumulation stays f32.
"""

import numpy as np

HEADS, WIN, DH, DR = 8, 16, 64, 32
B, C, H, W = 2, 256, 128, 128
NCORES = 8
SPOS = WIN * W          # positions per stripe = 2048
YW = W // WIN           # windows per stripe = 8
NU = 8                  # units per core: 2 stripes x 4
UP = 512                # positions per unit (2 windows)

_CACHE = {}


def _build():
    import bass_rust
    import concourse.bass as bass
    import concourse.mybir as mybir
    import concourse.tile as tile
    from contextlib import ExitStack

    f32 = mybir.dt.float32
    bf16 = mybir.dt.bfloat16
    AF = mybir.ActivationFunctionType
    MUL = mybir.AluOpType.mult
    ADD = mybir.AluOpType.add

    nc = bass.Bass("TRN2", target_bir_lowering=False, debug=False,
                   num_devices=NCORES)

    xs = nc.declare_dram_parameter("xs", [2, C, SPOS], bf16, isOutput=False)
    sks = nc.declare_dram_parameter("sks", [2, C, SPOS], bf16, isOutput=False)
    te = nc.declare_dram_parameter("te", [C, 1], f32, isOutput=False)
    cosR = nc.declare_dram_parameter("cosR", [2, 128, SPOS], bf16, isOutput=False)
    sinR = nc.declare_dram_parameter("sinR", [2, 128, SPOS], bf16, isOutput=False)
    wqm = nc.declare_dram_parameter("wqm", [C, 512], bf16, isOutput=False)
    wqr = nc.declare_dram_parameter("wqr", [C, 256], bf16, isOutput=False)
    wkm = nc.declare_dram_parameter("wkm", [C, 512], bf16, isOutput=False)
    wkr = nc.declare_dram_parameter("wkr", [C, 256], bf16, isOutput=False)
    wv = nc.declare_dram_parameter("wv", [C, 512], bf16, isOutput=False)
    wo = nc.declare_dram_parameter("wo", [512, C], bf16, isOutput=False)
    bo = nc.declare_dram_parameter("bo", [C, 1], f32, isOutput=False)
    out = nc.declare_dram_parameter("out", [2, C, SPOS], f32, isOutput=True)

    with tile.TileContext(nc) as tc:
        with ExitStack() as es:
            constp = es.enter_context(tc.tile_pool(name="const", bufs=1))
            xinp = es.enter_context(tc.tile_pool(name="xin", bufs=3))
            xbp = es.enter_context(tc.tile_pool(name="xb", bufs=2))
            csp = es.enter_context(tc.tile_pool(name="cs", bufs=3))
            qkp = es.enter_context(tc.tile_pool(name="qk", bufs=3))
            vslp = es.enter_context(tc.tile_pool(name="vsl", bufs=3))
            abp = es.enter_context(tc.tile_pool(name="ab", bufs=4))
            exp_ = es.enter_context(tc.tile_pool(name="ex", bufs=12))
            uwp = es.enter_context(tc.tile_pool(name="uw", bufs=3))
            rrp = es.enter_context(tc.tile_pool(name="rr", bufs=3))
            usp = es.enter_context(tc.tile_pool(name="us", bufs=3))
            osp = es.enter_context(tc.tile_pool(name="os", bufs=3))
            pproj = es.enter_context(tc.tile_pool(name="pproj", bufs=2, space="PSUM"))
            psim = es.enter_context(tc.tile_pool(name="psim", bufs=3, space="PSUM"))
            avp = es.enter_context(tc.tile_pool(name="avp", bufs=2, space="PSUM"))
            rbp = es.enter_context(tc.tile_pool(name="rbp", bufs=1, space="PSUM"))

            # ---------------- constants ----------------
            wq_i = [0]

            def wload(dram, cols, tag, nchunk=2):
                tiles = []
                for cx in range(nchunk):
                    t = constp.tile([128, cols], bf16, tag=f"{tag}{cx}", name=f"{tag}{cx}")
                    eng = nc.sync if wq_i[0] % 2 == 0 else nc.scalar
                    eng.dma_start(out=t[:], in_=dram[cx * 128:(cx + 1) * 128, :])
                    wq_i[0] += 1
                    tiles.append(t)
                return tiles

            wqm_t = wload(wqm, 512, "wqm")
            wqr_t = wload(wqr, 256, "wqr")
            wkm_t = wload(wkm, 512, "wkm")
            wkr_t = wload(wkr, 256, "wkr")
            wv_t = wload(wv, 512, "wv")
            wo_t = wload(wo, C, "wo", nchunk=4)

            te_t, bo_t = [], []
            for cx in range(2):
                t = constp.tile([128, 1], f32, tag=f"te{cx}", name=f"te{cx}")
                nc.sync.dma_start(out=t[:], in_=te[cx * 128:(cx + 1) * 128, :])
                te_t.append(t)
                t2 = constp.tile([128, 1], f32, tag=f"bo{cx}", name=f"bo{cx}")
                nc.sync.dma_start(out=t2[:], in_=bo[cx * 128:(cx + 1) * 128, :])
                bo_t.append(t2)

            ones64 = constp.tile([1, 64], bf16, tag="ones64")
            nc.vector.memset(ones64[:], 1.0)

            # per-unit state passed between skewed phases
            st = [None] * NU

            def loads(k):
                s, nt = k // 4, k % 4
                ntsl = slice(nt * UP, (nt + 1) * UP)
                xr = [xinp.tile([128, UP], bf16, tag=f"xr{cx}", name=f"xr{cx}") for cx in range(2)]
                skr = [xinp.tile([128, UP], bf16, tag=f"sk{cx}", name=f"sk{cx}") for cx in range(2)]
                for cx in range(2):
                    nc.sync.dma_start(out=xr[cx][:], in_=xs[s, cx * 128:(cx + 1) * 128, ntsl])
                    nc.sync.dma_start(out=skr[cx][:], in_=sks[s, cx * 128:(cx + 1) * 128, ntsl])
                cos_t = csp.tile([128, UP], bf16, tag="cos", name="cos_t")
                nc.sync.dma_start(out=cos_t[:], in_=cosR[s, :, ntsl])
                sin_t = csp.tile([128, UP], bf16, tag="sin", name="sin_t")
                nc.sync.dma_start(out=sin_t[:], in_=sinR[s, :, ntsl])
                st[k] = {"xr": xr, "skr": skr, "cos": cos_t, "sin": sin_t}

            def proj(k):
                u = st[k]
                # bias x with time-emb (bf16 SBUF, tensor_scalar -> fast mode)
                xb = [xbp.tile([128, UP], bf16, tag=f"xb{cx}", name=f"xb{cx}") for cx in range(2)]
                for cx in range(2):
                    nc.vector.tensor_scalar_add(xb[cx][:], u["xr"][cx][:], te_t[cx][:])
                u["xb"] = xb

                q_sl = [qkp.tile([128, UP], bf16, tag=f"q{i}", name=f"q{i}") for i in range(4)]
                kx_sl = [qkp.tile([128, UP], bf16, tag=f"kx{i}", name=f"kx{i}") for i in range(4)]
                ks_sl = [qkp.tile([128, UP], bf16, tag=f"ks{i}", name=f"ks{i}") for i in range(4)]
                u["q"], u["kx"], u["ks"] = q_sl, kx_sl, ks_sl
                ec = [0]

                def qkproj(wm, wr, src, dst):
                    # rot chunks C0/C1 + packed rot-weights R0/R1 -> combine
                    for cc in range(2):
                        pm = pproj.tile([128, UP], f32, tag="pp", name="pm")
                        for cx in range(2):
                            nc.tensor.matmul(pm[:], wm[cx][:, cc * 128:(cc + 1) * 128],
                                             src[cx][:], start=(cx == 0), stop=(cx == 1))
                        pr = pproj.tile([128, UP], f32, tag="pp", name="pr")
                        for cx in range(2):
                            nc.tensor.matmul(pr[:], wr[cx][:, cc * 128:(cc + 1) * 128],
                                             src[cx][:], start=(cx == 0), stop=(cx == 1))
                        a_t = abp.tile([128, UP], bf16, tag="ab", name="a_t")
                        nc.vector.tensor_tensor(out=a_t[:], in0=pm[:], in1=u["cos"][:], op=MUL)
                        b_t = abp.tile([128, UP], bf16, tag="ab", name="b_t")
                        nc.vector.tensor_tensor(out=b_t[:], in0=pr[:], in1=u["sin"][:], op=MUL)
                        nc.gpsimd.tensor_tensor(out=dst[cc][:], in0=a_t[:], in1=b_t[:], op=ADD)
                    # pass chunks C2/C3 -> plain evict (alternate ACT/DVE)
                    for cc in range(2):
                        pm = pproj.tile([128, UP], f32, tag="pp", name="pm")
                        for cx in range(2):
                            nc.tensor.matmul(pm[:], wm[cx][:, 256 + cc * 128:256 + (cc + 1) * 128],
                                             src[cx][:], start=(cx == 0), stop=(cx == 1))
                        if ec[0] % 2 == 0:
                            nc.scalar.copy(dst[2 + cc][:], pm[:])
                        else:
                            nc.vector.tensor_copy(dst[2 + cc][:], pm[:])
                        ec[0] += 1

                qkproj(wqm_t, wqr_t, xb, q_sl)
                qkproj(wkm_t, wkr_t, xb, kx_sl)
                qkproj(wkm_t, wkr_t, u["skr"], ks_sl)

                # V projection (transposed orientation), ones col per head
                vx_sl = [vslp.tile([128, 520], bf16, tag=f"vx{i}", name=f"vx{i}") for i in range(4)]
                vs_sl = [vslp.tile([128, 520], bf16, tag=f"vs{i}", name=f"vs{i}") for i in range(4)]
                u["vx"], u["vs"] = vx_sl, vs_sl
                for src, vdst in ((xb, vx_sl), (u["skr"], vs_sl)):
                    for pc in range(4):
                        psl = slice(pc * 128, (pc + 1) * 128)
                        pv = pproj.tile([128, UP], f32, tag="pp", name="pv")
                        for cx in range(2):
                            nc.tensor.matmul(pv[:], src[cx][:, psl], wv_t[cx][:],
                                             start=(cx == 0), stop=(cx == 1))
                        vt = vdst[pc]
                        dst = vt[:].rearrange("p (h c) -> p h c", h=8, c=65)
                        src_ap = pv[:].rearrange("p (h c) -> p h c", h=8, c=64)[:, :, :]
                        nc.scalar.copy(dst[:, :, 0:64], src_ap)
                        nc.gpsimd.memset(dst[:, :, 64:65], 1.0)

            def attn(k):
                u = st[k]
                q_sl, kx_sl, ks_sl = u["q"], u["kx"], u["ks"]
                uraw = uwp.tile([65, 8 * UP], bf16, tag="uraw", name="uraw")
                scale = float(DH) ** -0.5

                for trip in ((0, 1, 2), (3, 4, 5), (6, 7)):
                    expt = {}
                    for wy in range(2):
                        for half in range(2):      # 0 = x branch, 1 = skip branch
                            k_sl = kx_sl if half == 0 else ks_sl
                            sims = {}
                            for h in trip:
                                sims[h] = psim.tile([128, UP], f32, tag="sim", name=f"sim{h}")
                            for kc in range(2):
                                for part in range(2):   # 0 rot, 1 pass
                                    for h in trip:
                                        ci = (h // 4) + 2 * part
                                        r0 = 32 * (h % 4)
                                        lhsT = k_sl[ci][r0:r0 + 32,
                                                        wy * 256 + kc * 128: wy * 256 + kc * 128 + 128]
                                        rhs = q_sl[ci][r0:r0 + 32, wy * 256: wy * 256 + 256]
                                        nc.tensor.matmul(
                                            sims[h][:, kc * 256:(kc + 1) * 256], lhsT, rhs,
                                            start=(part == 0), stop=(part == 1),
                                            tile_position=(r0, 0))
                            for h in trip:
                                et = exp_.tile([128, UP], bf16, tag="exp", name="et")
                                nc.scalar.activation(et[:], sims[h][:], AF.Exp, scale=scale)
                                expt[(h, wy, half)] = et
                    for h in trip:
                        av = avp.tile([65, UP], f32, tag="av", name="av")
                        for wy in range(2):
                            for mc in range(4):
                                vt = (u["vx"] if mc < 2 else u["vs"])[wy * 2 + (mc % 2)]
                                et = expt[(h, wy, mc // 2)]
                                nc.tensor.matmul(av[:, wy * 256:(wy + 1) * 256],
                                                 vt[:, h * 65: h * 65 + 65],
                                                 et[:, (mc % 2) * 256:(mc % 2) * 256 + 256],
                                                 start=(mc == 0), stop=(mc == 3))
                        nc.vector.tensor_copy(uraw[:, h * UP:(h + 1) * UP], av[:])

                # denominators: row 64 -> [8, 512] staging -> reciprocal -> row
                rdr = rrp.tile([8, UP], bf16, tag="rdr", name="rdr")
                nc.sync.dma_start(out=rdr[:], in_=uraw[64:65, :])
                rin = rrp.tile([8, UP], bf16, tag="rin", name="rin")
                with nc.allow_low_precision(reason="softmax denominator reciprocal"):
                    nc.vector.reciprocal(rin[:], rdr[:])
                runit = rrp.tile([1, 8 * UP], bf16, tag="runit", name="runit")
                nc.sync.dma_start(out=runit[:], in_=rin[:])

                us = [usp.tile([128, UP], bf16, tag=f"u{i}", name=f"u{i}") for i in range(4)]
                u["us"] = us
                for h in range(8):
                    rb = rbp.tile([64, UP], f32, tag="rb", name="rb")
                    nc.tensor.matmul(rb[:], ones64[:], runit[0:1, h * UP:(h + 1) * UP],
                                     start=True, stop=True)
                    nc.vector.tensor_tensor(
                        out=us[h // 2][64 * (h % 2):64 * (h % 2) + 64, :],
                        in0=uraw[0:64, h * UP:(h + 1) * UP], in1=rb[:], op=MUL)

            def outproj(k):
                u = st[k]
                s, nt = k // 4, k % 4
                ntsl = slice(nt * UP, (nt + 1) * UP)
                for m2 in range(2):
                    fp = pproj.tile([128, UP], f32, tag="pp", name="fp")
                    for ic in range(4):
                        nc.tensor.matmul(fp[:], wo_t[ic][:, m2 * 128:(m2 + 1) * 128],
                                         u["us"][ic][:], start=(ic == 0), stop=(ic == 3))
                    osb = osp.tile([128, UP], f32, tag="ot", name="osb")
                    nc.vector.tensor_scalar_add(osb[:], fp[:], bo_t[m2][:])
                    nc.sync.dma_start(out=out[s, m2 * 128:(m2 + 1) * 128, ntsl], in_=osb[:])
                st[k] = None

            loads(0)
            for k in range(NU + 2):
                if k + 1 < NU:
                    loads(k + 1)
                if k < NU:
                    proj(k)
                if k >= 1 and k - 1 < NU:
                    attn(k - 1)
                if k >= 2:
                    outproj(k - 2)

    _split_excess_waits(nc, bass_rust, mybir)
    return nc


def _split_excess_waits(nc, bass_rust, mybir, max_waits=1):
    """walrus in this toolchain accepts one sync-wait command per instruction;
    hoist excess waits onto same-engine NoOps inserted just before."""
    n_added = 0
    for f in nc.m.functions:
        for bb in f.blocks:
            insts = list(bb.instructions)
            new = []
            dirty = False
            for inst in insts:
                si = inst.sync_info
                if si is not None and len(si.on_wait) > max_waits:
                    waits = list(si.on_wait)
                    for wt in waits[:-max_waits]:
                        nop = mybir.InstNoOp(name=f"{inst.name}-ws{n_added}", ins=[], outs=[])
                        nop.engine = inst.engine
                        nop.sync_info = bass_rust.SyncInfo(on_wait=[wt], on_update=[])
                        new.append(nop)
                        n_added += 1
                    inst.sync_info = bass_rust.SyncInfo(
                        on_wait=waits[-max_waits:], on_update=list(si.on_update))
                    dirty = True
                new.append(inst)
            if dirty:
                bb.instructions = new
    return n_added


def _window_major(a):
    """[..., 16, 128] spatial block -> [..., 2048] window-major positions."""
    lead = a.shape[:-2]
    return (a.reshape(*lead, WIN, YW, WIN)
             .swapaxes(-3, -2)
             .reshape(*lead, SPOS))


def _host_weights(Wq, Wkv, Wout, bout, time_emb):
    import ml_dtypes
    bf = ml_dtypes.bfloat16
    Wk_, Wv_ = Wkv[:512], Wkv[512:]

    # C-layout: [C0 C1] rot channels (h*32+i <- 64h+i), [C2 C3] pass channels
    perm = ([64 * h + i for h in range(HEADS) for i in range(DR)] +
            [64 * h + DR + i for h in range(HEADS) for i in range(DR)])

    def rot_pack(Wm):
        R = np.empty((256, C), np.float32)
        for h in range(HEADS):
            for i in range(DR):
                if i % 2 == 0:
                    R[32 * h + i] = -Wm[64 * h + i + 1]
                else:
                    R[32 * h + i] = Wm[64 * h + i - 1]
        return R

    return {
        "wqm": np.ascontiguousarray(Wq[perm].T).astype(bf),
        "wqr": np.ascontiguousarray(rot_pack(Wq).T).astype(bf),
        "wkm": np.ascontiguousarray(Wk_[perm].T).astype(bf),
        "wkr": np.ascontiguousarray(rot_pack(Wk_).T).astype(bf),
        "wv": np.ascontiguousarray(Wv_.T).astype(bf),
        "wo": np.ascontiguousarray(Wout.T).astype(bf),
        "bo": bout.reshape(C, 1).astype(np.float32),
    }


def _make_core_inputs(x, skip, time_emb, sin, cos, Wq, Wkv, Wout, bout):
    import ml_dtypes
    bf = ml_dtypes.bfloat16
    wmap = _host_weights(Wq, Wkv, Wout, bout, time_emb)

    in_maps = []
    for core in range(NCORES):
        b = core // 4
        xbs = [2 * (core % 4), 2 * (core % 4) + 1]
        xs_c = np.empty((2, C, SPOS), bf)
        sk_c = np.empty((2, C, SPOS), bf)
        cos_c = np.empty((2, 128, SPOS), bf)
        sin_c = np.empty((2, 128, SPOS), bf)
        for si, xb in enumerate(xbs):
            rs = slice(xb * WIN, (xb + 1) * WIN)
            xs_c[si] = _window_major(x[b, :, rs, :])
            sk_c[si] = _window_major(skip[b, :, rs, :])
            cw = _window_major(cos[rs].transpose(2, 0, 1))   # [32, 2048]
            sw = _window_major(sin[rs].transpose(2, 0, 1))
            for h4 in range(4):
                cos_c[si, h4 * 32:(h4 + 1) * 32] = cw
                sin_c[si, h4 * 32:(h4 + 1) * 32] = sw
        m = {"xs": xs_c, "sks": sk_c,
             "te": time_emb[b].reshape(C, 1).astype(np.float32),
             "cosR": cos_c, "sinR": sin_c}
        m.update(wmap)
        in_maps.append(m)
    return in_maps


def _assemble(results):
    out_full = np.empty((B, C, H, W), np.float32)
    for core in range(NCORES):
        b = core // 4
        xbs = [2 * (core % 4), 2 * (core % 4) + 1]
        o = results[core]["out"]          # [2, C, 2048] window-major
        for si, xb in enumerate(xbs):
            blk = (np.asarray(o[si], np.float32).reshape(C, YW, WIN, WIN)
                        .swapaxes(1, 2)
                        .reshape(C, WIN, W))
            out_full[b, :, xb * WIN:(xb + 1) * WIN, :] = blk
    return out_full


def get_nc():
    if "nc" not in _CACHE:
        _CACHE["nc"] = _build()
    return _CACHE["nc"]


def kernel(x, skip, time_emb, sin, cos, Wq, Wkv, Wout, bout):
    from concourse.bass_utils import run_bass_kernel_spmd
    args = [np.asarray(a, dtype=np.float32) for a in
            (x, skip, time_emb, sin, cos, Wq, Wkv, Wout, bout)]
    nc = get_nc()
    in_maps = _make_core_inputs(*args)
    res = run_bass_kernel_spmd(nc, in_maps, list(range(NCORES)), trace=False)
    return _assemble(res.results)


# revision 19
# speedup vs baseline: 1.0085x; 1.0085x over previous
"""Trainium2 Bass kernel for nn_Attention_6201932775733 (sparse window attention).

v2 design (8 NeuronCores, SPMD, no collectives):
  - Data-parallel over (batch, 16-row stripe blocks): core i handles batch
    i//4, x-blocks {2*(i%4), 2*(i%4)+1}; positions pre-permuted window-major.
  - Work is pipelined over 8 "units" per core (stripe x 512-position block =
    2 windows x 8 heads), with projection(k), attention(k-1), out-proj(k-2)
    emitted skewed so all engines overlap.
  - Q/K channel layout is split rot/pass: chunks [C0 C1] = rotary channels
    (heads 0-3 / 4-7, 32 each), [C2 C3] = pass-through channels. Pass chunks
    evict PSUM->SBUF with a plain copy; only rot chunks pay the cos/sin
    multiply-add. RoPE rotation weights are packed (no zero rows).
  - sim per (window, head) = K=32 matmuls (rot + pass accumulate); heads in
    a triple target distinct 32-row PE groups so their matmuls overlap.
  - AV uses a ones-column per head so the softmax denominator is row 64 of
    the [65, 512] window-pair PSUM tile; denominators round-trip through two
    reshaping DMAs for a partition-parallel reciprocal, then a K=1 matmul
    broadcasts 1/d and one DVE multiply normalizes into the bf16 u slab.
  - Everything on the PE path is bf16 (host pre-converts inputs/weights);
    PSUM accumulation stays f32.
"""

import numpy as np

HEADS, WIN, DH, DR = 8, 16, 64, 32
B, C, H, W = 2, 256, 128, 128
NCORES = 8
SPOS = WIN * W          # positions per stripe = 2048
YW = W // WIN           # windows per stripe = 8
NU = 8                  # units per core: 2 stripes x 4
UP = 512                # positions per unit (2 windows)

_CACHE = {}


def _build():
    import bass_rust
    import concourse.bass as bass
    import concourse.mybir as mybir
    import concourse.tile as tile
    from contextlib import ExitStack

    f32 = mybir.dt.float32
    bf16 = mybir.dt.bfloat16
    AF = mybir.ActivationFunctionType
    MUL = mybir.AluOpType.mult
    ADD = mybir.AluOpType.add

    nc = bass.Bass("TRN2", target_bir_lowering=False, debug=False,
                   num_devices=NCORES)

    xs = nc.declare_dram_parameter("xs", [2, C, SPOS], bf16, isOutput=False)
    sks = nc.declare_dram_parameter("sks", [2, C, SPOS], bf16, isOutput=False)
    te = nc.declare_dram_parameter("te", [C, 1], f32, isOutput=False)
    cosR = nc.declare_dram_parameter("cosR", [2, 128, SPOS], bf16, isOutput=False)
    sinR = nc.declare_dram_parameter("sinR", [2, 128, SPOS], bf16, isOutput=False)
    wqm = nc.declare_dram_parameter("wqm", [C, 512], bf16, isOutput=False)
    wqr = nc.declare_dram_parameter("wqr", [C, 256], bf16, isOutput=False)
    wkm = nc.declare_dram_parameter("wkm", [C, 512], bf16, isOutput=False)
    wkr = nc.declare_dram_parameter("wkr", [C, 256], bf16, isOutput=False)
    wv = nc.declare_dram_parameter("wv", [C, 512], bf16, isOutput=False)
    wo = nc.declare_dram_parameter("wo", [512, C], bf16, isOutput=False)
    bo = nc.declare_dram_parameter("bo", [C, 1], f32, isOutput=False)
    out = nc.declare_dram_parameter("out", [2, C, SPOS], f32, isOutput=True)

    with tile.TileContext(nc) as tc:
        with ExitStack() as es:
            constp = es.enter_context(tc.tile_pool(name="const", bufs=1))
            xinp = es.enter_context(tc.tile_pool(name="xin", bufs=3))
            xbp = es.enter_context(tc.tile_pool(name="xb", bufs=2))
            csp = es.enter_context(tc.tile_pool(name="cs", bufs=3))
            qkp = es.enter_context(tc.tile_pool(name="qk", bufs=3))
            vslp = es.enter_context(tc.tile_pool(name="vsl", bufs=3))
            abp = es.enter_context(tc.tile_pool(name="ab", bufs=4))
            exp_ = es.enter_context(tc.tile_pool(name="ex", bufs=12))
            uwp = es.enter_context(tc.tile_pool(name="uw", bufs=3))
            rrp = es.enter_context(tc.tile_pool(name="rr", bufs=3))
            usp = es.enter_context(tc.tile_pool(name="us", bufs=3))
            osp = es.enter_context(tc.tile_pool(name="os", bufs=3))
            pproj = es.enter_context(tc.tile_pool(name="pproj", bufs=2, space="PSUM"))
            psim = es.enter_context(tc.tile_pool(name="psim", bufs=3, space="PSUM"))
            avp = es.enter_context(tc.tile_pool(name="avp", bufs=2, space="PSUM"))
            rbp = es.enter_context(tc.tile_pool(name="rbp", bufs=1, space="PSUM"))

            # ---------------- constants ----------------
            wq_i = [0]

            def wload(dram, cols, tag, nchunk=2):
                tiles = []
                for cx in range(nchunk):
                    t = constp.tile([128, cols], bf16, tag=f"{tag}{cx}", name=f"{tag}{cx}")
                    nc.scalar.dma_start(out=t[:], in_=dram[cx * 128:(cx + 1) * 128, :])
                    tiles.append(t)
                return tiles

            wqm_t = wload(wqm, 512, "wqm")
            wqr_t = wload(wqr, 256, "wqr")
            wkm_t = wload(wkm, 512, "wkm")
            wkr_t = wload(wkr, 256, "wkr")
            wv_t = wload(wv, 512, "wv")
            wo_t = wload(wo, C, "wo", nchunk=4)

            te_t, bo_t = [], []
            for cx in range(2):
                t = constp.tile([128, 1], f32, tag=f"te{cx}", name=f"te{cx}")
                nc.sync.dma_start(out=t[:], in_=te[cx * 128:(cx + 1) * 128, :])
                te_t.append(t)
                t2 = constp.tile([128, 1], f32, tag=f"bo{cx}", name=f"bo{cx}")
                nc.sync.dma_start(out=t2[:], in_=bo[cx * 128:(cx + 1) * 128, :])
                bo_t.append(t2)

            ones64 = constp.tile([1, 64], bf16, tag="ones64")
            nc.vector.memset(ones64[:], 1.0)

            # per-unit state passed between skewed phases
            st = [None] * NU

            def loads(k):
                s, nt = k // 4, k % 4
                ntsl = slice(nt * UP, (nt + 1) * UP)
                xr = [xinp.tile([128, UP], bf16, tag=f"xr{cx}", name=f"xr{cx}") for cx in range(2)]
                skr = [xinp.tile([128, UP], bf16, tag=f"sk{cx}", name=f"sk{cx}") for cx in range(2)]
                for cx in range(2):
                    nc.sync.dma_start(out=xr[cx][:], in_=xs[s, cx * 128:(cx + 1) * 128, ntsl])
                    nc.sync.dma_start(out=skr[cx][:], in_=sks[s, cx * 128:(cx + 1) * 128, ntsl])
                cos_t = csp.tile([128, UP], bf16, tag="cos", name="cos_t")
                nc.sync.dma_start(out=cos_t[:], in_=cosR[s, :, ntsl])
                sin_t = csp.tile([128, UP], bf16, tag="sin", name="sin_t")
                nc.sync.dma_start(out=sin_t[:], in_=sinR[s, :, ntsl])
                st[k] = {"xr": xr, "skr": skr, "cos": cos_t, "sin": sin_t}

            def proj(k):
                u = st[k]
                # bias x with time-emb (bf16 SBUF, tensor_scalar -> fast mode)
                xb = [xbp.tile([128, UP], bf16, tag=f"xb{cx}", name=f"xb{cx}") for cx in range(2)]
                for cx in range(2):
                    nc.vector.tensor_scalar_add(xb[cx][:], u["xr"][cx][:], te_t[cx][:])
                u["xb"] = xb

                q_sl = [qkp.tile([128, UP], bf16, tag=f"q{i}", name=f"q{i}") for i in range(4)]
                kx_sl = [qkp.tile([128, UP], bf16, tag=f"kx{i}", name=f"kx{i}") for i in range(4)]
                ks_sl = [qkp.tile([128, UP], bf16, tag=f"ks{i}", name=f"ks{i}") for i in range(4)]
                u["q"], u["kx"], u["ks"] = q_sl, kx_sl, ks_sl
                ec = [0]

                def qkproj(wm, wr, src, dst):
                    # rot chunks C0/C1 + packed rot-weights R0/R1 -> combine
                    for cc in range(2):
                        pm = pproj.tile([128, UP], f32, tag="pp", name="pm")
                        for cx in range(2):
                            nc.tensor.matmul(pm[:], wm[cx][:, cc * 128:(cc + 1) * 128],
                                             src[cx][:], start=(cx == 0), stop=(cx == 1))
                        pr = pproj.tile([128, UP], f32, tag="pp", name="pr")
                        for cx in range(2):
                            nc.tensor.matmul(pr[:], wr[cx][:, cc * 128:(cc + 1) * 128],
                                             src[cx][:], start=(cx == 0), stop=(cx == 1))
                        a_t = abp.tile([128, UP], bf16, tag="ab", name="a_t")
                        nc.vector.tensor_tensor(out=a_t[:], in0=pm[:], in1=u["cos"][:], op=MUL)
                        b_t = abp.tile([128, UP], bf16, tag="ab", name="b_t")
                        nc.vector.tensor_tensor(out=b_t[:], in0=pr[:], in1=u["sin"][:], op=MUL)
                        nc.gpsimd.tensor_tensor(out=dst[cc][:], in0=a_t[:], in1=b_t[:], op=ADD)
                    # pass chunks C2/C3 -> plain evict (alternate ACT/DVE)
                    for cc in range(2):
                        pm = pproj.tile([128, UP], f32, tag="pp", name="pm")
                        for cx in range(2):
                            nc.tensor.matmul(pm[:], wm[cx][:, 256 + cc * 128:256 + (cc + 1) * 128],
                                             src[cx][:], start=(cx == 0), stop=(cx == 1))
                        if ec[0] % 2 == 0:
                            nc.scalar.copy(dst[2 + cc][:], pm[:])
                        else:
                            nc.vector.tensor_copy(dst[2 + cc][:], pm[:])
                        ec[0] += 1

                qkproj(wqm_t, wqr_t, xb, q_sl)
                qkproj(wkm_t, wkr_t, xb, kx_sl)
                qkproj(wkm_t, wkr_t, u["skr"], ks_sl)

                # V projection (transposed orientation), ones col per head
                vx_sl = [vslp.tile([128, 520], bf16, tag=f"vx{i}", name=f"vx{i}") for i in range(4)]
                vs_sl = [vslp.tile([128, 520], bf16, tag=f"vs{i}", name=f"vs{i}") for i in range(4)]
                u["vx"], u["vs"] = vx_sl, vs_sl
                for src, vdst in ((xb, vx_sl), (u["skr"], vs_sl)):
                    for pc in range(4):
                        psl = slice(pc * 128, (pc + 1) * 128)
                        pv = pproj.tile([128, UP], f32, tag="pp", name="pv")
                        for cx in range(2):
                            nc.tensor.matmul(pv[:], src[cx][:, psl], wv_t[cx][:],
                                             start=(cx == 0), stop=(cx == 1))
                        vt = vdst[pc]
                        dst = vt[:].rearrange("p (h c) -> p h c", h=8, c=65)
                        src_ap = pv[:].rearrange("p (h c) -> p h c", h=8, c=64)[:, :, :]
                        nc.scalar.copy(dst[:, :, 0:64], src_ap)
                        nc.gpsimd.memset(dst[:, :, 64:65], 1.0)

            def attn(k):
                u = st[k]
                q_sl, kx_sl, ks_sl = u["q"], u["kx"], u["ks"]
                uraw = uwp.tile([65, 8 * UP], bf16, tag="uraw", name="uraw")
                scale = float(DH) ** -0.5

                for trip in ((0, 1, 2), (3, 4, 5), (6, 7)):
                    expt = {}
                    for wy in range(2):
                        for half in range(2):      # 0 = x branch, 1 = skip branch
                            k_sl = kx_sl if half == 0 else ks_sl
                            sims = {}
                            for h in trip:
                                sims[h] = psim.tile([128, UP], f32, tag="sim", name=f"sim{h}")
                            for kc in range(2):
                                for part in range(2):   # 0 rot, 1 pass
                                    for h in trip:
                                        ci = (h // 4) + 2 * part
                                        r0 = 32 * (h % 4)
                                        lhsT = k_sl[ci][r0:r0 + 32,
                                                        wy * 256 + kc * 128: wy * 256 + kc * 128 + 128]
                                        rhs = q_sl[ci][r0:r0 + 32, wy * 256: wy * 256 + 256]
                                        nc.tensor.matmul(
                                            sims[h][:, kc * 256:(kc + 1) * 256], lhsT, rhs,
                                            start=(part == 0), stop=(part == 1),
                                            tile_position=(r0, 0))
                            for h in trip:
                                et = exp_.tile([128, UP], bf16, tag="exp", name="et")
                                nc.scalar.activation(et[:], sims[h][:], AF.Exp, scale=scale)
                                expt[(h, wy, half)] = et
                    for h in trip:
                        av = avp.tile([65, UP], f32, tag="av", name="av")
                        for wy in range(2):
                            for mc in range(4):
                                vt = (u["vx"] if mc < 2 else u["vs"])[wy * 2 + (mc % 2)]
                                et = expt[(h, wy, mc // 2)]
                                nc.tensor.matmul(av[:, wy * 256:(wy + 1) * 256],
                                                 vt[:, h * 65: h * 65 + 65],
                                                 et[:, (mc % 2) * 256:(mc % 2) * 256 + 256],
                                                 start=(mc == 0), stop=(mc == 3))
                        nc.vector.tensor_copy(uraw[:, h * UP:(h + 1) * UP], av[:])

                # denominators: row 64 -> [8, 512] staging -> reciprocal -> row
                rdr = rrp.tile([8, UP], bf16, tag="rdr", name="rdr")
                nc.sync.dma_start(out=rdr[:], in_=uraw[64:65, :])
                rin = rrp.tile([8, UP], bf16, tag="rin", name="rin")
                with nc.allow_low_precision(reason="softmax denominator reciprocal"):
                    nc.vector.reciprocal(rin[:], rdr[:])
                runit = rrp.tile([1, 8 * UP], bf16, tag="runit", name="runit")
                nc.sync.dma_start(out=runit[:], in_=rin[:])

                us = [usp.tile([128, UP], bf16, tag=f"u{i}", name=f"u{i}") for i in range(4)]
                u["us"] = us
                for h in range(8):
                    rb = rbp.tile([64, UP], f32, tag="rb", name="rb")
                    nc.tensor.matmul(rb[:], ones64[:], runit[0:1, h * UP:(h + 1) * UP],
                                     start=True, stop=True)
                    nc.vector.tensor_tensor(
                        out=us[h // 2][64 * (h % 2):64 * (h % 2) + 64, :],
                        in0=uraw[0:64, h * UP:(h + 1) * UP], in1=rb[:], op=MUL)

            def outproj(k):
                u = st[k]
                s, nt = k // 4, k % 4
                ntsl = slice(nt * UP, (nt + 1) * UP)
                for m2 in range(2):
                    fp = pproj.tile([128, UP], f32, tag="pp", name="fp")
                    for ic in range(4):
                        nc.tensor.matmul(fp[:], wo_t[ic][:, m2 * 128:(m2 + 1) * 128],
                                         u["us"][ic][:], start=(ic == 0), stop=(ic == 3))
                    osb = osp.tile([128, UP], f32, tag="ot", name="osb")
                    nc.vector.tensor_scalar_add(osb[:], fp[:], bo_t[m2][:])
                    nc.sync.dma_start(out=out[s, m2 * 128:(m2 + 1) * 128, ntsl], in_=osb[:])
                st[k] = None

            loads(0)
            for k in range(NU + 2):
                if k + 1 < NU:
                    loads(k + 1)
                if k < NU:
                    proj(k)
                if k >= 1 and k - 1 < NU:
                    attn(k - 1)
                if k >= 2:
                    outproj(k - 2)

    _split_excess_waits(nc, bass_rust, mybir)
    return nc


def _split_excess_waits(nc, bass_rust, mybir, max_waits=1):
    """walrus in this toolchain accepts one sync-wait command per instruction;
    hoist excess waits onto same-engine NoOps inserted just before."""
    n_added = 0
    for f in nc.m.functions:
        for bb in f.blocks:
            insts = list(bb.instructions)
            new = []
            dirty = False
            for inst in insts:
                si = inst.sync_info
                if si is not None and len(si.on_wait) > max_waits:
                    waits = list(si.on_wait)
                    for wt in waits[:-max_waits]:
                        nop = mybir.InstNoOp(name=f"{inst.name}-ws{n_added}", ins=[], outs=[])
                        nop.engine = inst.engine
                        nop.sync_info = bass_rust.SyncInfo(on_wait=[wt], on_update=[])
                        new.append(nop)
                        n_added += 1
                    inst.sync_info = bass_rust.SyncInfo(
                        on_wait=waits[-max_waits:], on_update=list(si.on_update))
                    dirty = True
                new.append(inst)
            if dirty:
                bb.instructions = new
    return n_added


def _window_major(a):
    """[..., 16, 128] spatial block -> [..., 2048] window-major positions."""
    lead = a.shape[:-2]
    return (a.reshape(*lead, WIN, YW, WIN)
             .swapaxes(-3, -2)
             .reshape(*lead, SPOS))


def _host_weights(Wq, Wkv, Wout, bout, time_emb):
    import ml_dtypes
    bf = ml_dtypes.bfloat16
    Wk_, Wv_ = Wkv[:512], Wkv[512:]

    # C-layout: [C0 C1] rot channels (h*32+i <- 64h+i), [C2 C3] pass channels
    perm = ([64 * h + i for h in range(HEADS) for i in range(DR)] +
            [64 * h + DR + i for h in range(HEADS) for i in range(DR)])

    def rot_pack(Wm):
        R = np.empty((256, C), np.float32)
        for h in range(HEADS):
            for i in range(DR):
                if i % 2 == 0:
                    R[32 * h + i] = -Wm[64 * h + i + 1]
                else:
                    R[32 * h + i] = Wm[64 * h + i - 1]
        return R

    return {
        "wqm": np.ascontiguousarray(Wq[perm].T).astype(bf),
        "wqr": np.ascontiguousarray(rot_pack(Wq).T).astype(bf),
        "wkm": np.ascontiguousarray(Wk_[perm].T).astype(bf),
        "wkr": np.ascontiguousarray(rot_pack(Wk_).T).astype(bf),
        "wv": np.ascontiguousarray(Wv_.T).astype(bf),
        "wo": np.ascontiguousarray(Wout.T).astype(bf),
        "bo": bout.reshape(C, 1).astype(np.float32),
    }


def _make_core_inputs(x, skip, time_emb, sin, cos, Wq, Wkv, Wout, bout):
    import ml_dtypes
    bf = ml_dtypes.bfloat16
    wmap = _host_weights(Wq, Wkv, Wout, bout, time_emb)

    in_maps = []
    for core in range(NCORES):
        b = core // 4
        xbs = [2 * (core % 4), 2 * (core % 4) + 1]
        xs_c = np.empty((2, C, SPOS), bf)
        sk_c = np.empty((2, C, SPOS), bf)
        cos_c = np.empty((2, 128, SPOS), bf)
        sin_c = np.empty((2, 128, SPOS), bf)
        for si, xb in enumerate(xbs):
            rs = slice(xb * WIN, (xb + 1) * WIN)
            xs_c[si] = _window_major(x[b, :, rs, :])
            sk_c[si] = _window_major(skip[b, :, rs, :])
            cw = _window_major(cos[rs].transpose(2, 0, 1))   # [32, 2048]
            sw = _window_major(sin[rs].transpose(2, 0, 1))
            for h4 in range(4):
                cos_c[si, h4 * 32:(h4 + 1) * 32] = cw
                sin_c[si, h4 * 32:(h4 + 1) * 32] = sw
        m = {"xs": xs_c, "sks": sk_c,
             "te": time_emb[b].reshape(C, 1).astype(np.float32),
             "cosR": cos_c, "sinR": sin_c}
        m.update(wmap)
        in_maps.append(m)
    return in_maps


def _assemble(results):
    out_full = np.empty((B, C, H, W), np.float32)
    for core in range(NCORES):
        b = core // 4
        xbs = [2 * (core % 4), 2 * (core % 4) + 1]
        o = results[core]["out"]          # [2, C, 2048] window-major
        for si, xb in enumerate(xbs):
            blk = (np.asarray(o[si], np.float32).reshape(C, YW, WIN, WIN)
                        .swapaxes(1, 2)
                        .reshape(C, WIN, W))
            out_full[b, :, xb * WIN:(xb + 1) * WIN, :] = blk
    return out_full


def get_nc():
    if "nc" not in _CACHE:
        _CACHE["nc"] = _build()
    return _CACHE["nc"]


def kernel(x, skip, time_emb, sin, cos, Wq, Wkv, Wout, bout):
    from concourse.bass_utils import run_bass_kernel_spmd
    args = [np.asarray(a, dtype=np.float32) for a in
            (x, skip, time_emb, sin, cos, Wq, Wkv, Wout, bout)]
    nc = get_nc()
    in_maps = _make_core_inputs(*args)
    res = run_bass_kernel_spmd(nc, in_maps, list(range(NCORES)), trace=False)
    return _assemble(res.results)
